# revision 1
# baseline (speedup 1.0000x reference)
"""Trainium2 Bass kernel for CrossModalAttention.

Reference computation (B=1, C=64, N=8192 voxels):
  two cross-attention directions (CT queries over MRI keys/values and vice
  versa), each with an 8192x8192 attention matrix, fused output projection.

Sharding: each of the 8 cores owns 1024 query voxels for BOTH directions,
computes K/V over the full sequence locally (features are only 2 MB per
modality), and produces its own (64, 1024) slice of the output through the
final projection. No collectives; the host concatenates the 8 slices.

Device algorithm ("transposed world", no transposes of large tensors):
  The K projection is folded into the query side (associativity:
  (Wk f)^T q = f^T (Wk^T q)), so scores read the fp16 features directly:
    scores^T (j,i) = matmul(lhsT=feat_aug[:, j-chunk] (65,128),
                            rhs=q''_d (65,512)),  q''_d = [Wk^T q_d; bk.q_d]
  exp on ScalarE straight out of PSUM (max-subtraction skipped: |s| <= ~1.2),
  batched 3 PSUM banks per ACTIVATE to amortize the 352-cycle overhead.
  AV is flipped to out=(i,c) so the PE output partition dim is the full 128:
    att^T[i, 0:65] += matmul(lhsT=exp chunk (j,128i), rhs=V^T_aug (j,65))
  V^T_aug = feat_aug^T @ Wv' where Wv' carries the bias row and a final
  [0..0,1] column, so column 64 of att^T accumulates the softmax
  denominator for free. All four 128-query chains of one 512-query block
  share a single PSUM bank: only the first matmul uses start=True (clears
  the bank); later chains' first writes land on has_written=0 cells and
  overwrite, which initializes them correctly.
  Normalize = per-partition reciprocal of column 64 + tensor_scalar mult,
  then a 128x64 PE transpose per subblock rebuilds the channel-major
  fused tile for the fp32 final projection.

Precision: matmul operands are fp16 (PE streams 1 col/cycle with fast
weight load; fp32 is 4x slower, float32r 2x, and bf16's 8-bit mantissa
loses 10x accuracy for identical speed -- all values here are far inside
fp16 range). Accumulation is always fp32 in PSUM; softmax denominator,
normalization, transposes and the final projection are fp32. Rounding
errors of q/k/exp/v average out over the 8192-key softmax: end-to-end
error ~2e-4.
"""

from contextlib import ExitStack

import numpy as np

import concourse.bass as bass
import concourse.mybir as mybir
import concourse.tile as tile
from concourse import bacc
from concourse.bass_utils import run_bass_kernel_spmd

F32 = mybir.dt.float32
F16 = mybir.dt.float16
C = 64          # channels
N = 8192        # voxels (8*32*32)
NCORES = 8
NQ = N // NCORES      # 1024 queries per core
IH = 512              # query block (PSUM bank width in f32)
NIH = NQ // IH        # 2
NSUB = IH // 128      # 4 query subblocks per block (AV lhsT width)
JCH = 128             # key chunk (AV contraction tile)
NJ = N // JCH         # 64
GRP = 2               # score banks per exp group (2 banks -> one wide ACT op;
                      # 3 slots of 2 banks let AV lag behind scores so the
                      # PE stream never waits on ScalarE)
LAG = 1               # groups the AV matmuls trail the score matmuls by
VGW = 4               # vT chunks per projection group (4*65 f32 fits one bank)
W = C + 1             # 65: augmented channel dim
VCOLS = NJ * W        # vT storage: 64 chunks x 65 cols (65th col = denom ones)
NFS = 8               # feature DMA subtiles
FSW = N // NFS        # 1024 cols per subtile
JPS = FSW // JCH      # 8 j-chunks per feature subtile


def _emit_feat_load(nc, featp, feat_dram, tag, name):
    subs = []
    for s in range(NFS):
        t = featp.tile([W, FSW], F16, tag=tag, name=f"{name}{s}")
        nc.sync.dma_start(t[:], feat_dram[:, FSW * s : FSW * (s + 1)])
        subs.append(t)
    return subs


def _emit_qq_proj(nc, pools, wqq_sb, qsrc, wcol, name):
    """q''_d (65, NQ) = (Wq_aug @ [Wk|bk])^T @ qsrc_aug -- the Q projection and
    the K projection (folded onto the query side) composed on the host."""
    qp, sp = pools["qp"], pools["sp"]
    qq = qp.tile([W, NQ], F16, tag="qq", name=name)
    for h in range(NIH):
        pq = sp.tile([W, IH], F32, tag="ps", name=f"pqq_{name}{h}")
        nc.tensor.matmul(
            pq[:],
            lhsT=wqq_sb[:, wcol : wcol + W],
            rhs=qsrc[:, IH * h : IH * (h + 1)],
            start=True,
            stop=True,
        )
        nc.vector.tensor_copy(qq[:, IH * h : IH * (h + 1)], pq[:])
    return qq


def _emit_v_proj(nc, pools, wv_sb, fs, wcol, name):
    """vT_aug (128j x 65) chunks = feat_aug^T @ Wv' (ones column built in)."""
    vp, sp = pools["vp"], pools["sp"]
    vT = vp.tile([JCH, VCOLS], F16, tag="v", name=name)
    for g in range(NJ // VGW):
        pv = sp.tile([JCH, VGW * W], F32, tag="ps", name=f"pv_{name}{g}")
        for cc in range(VGW):
            j = VGW * g + cc
            nc.tensor.matmul(
                pv[:, W * cc : W * (cc + 1)],
                lhsT=fs[j // JPS][:, JCH * (j % JPS) : JCH * (j % JPS + 1)],
                rhs=wv_sb[:, wcol : wcol + W],
                start=True,
                stop=True,
            )
        nc.vector.tensor_copy(vT[:, W * VGW * g : W * VGW * (g + 1)], pv[:])
    return vT


def _emit_attention(nc, pools, fa, qq, vT, fused_t, d):
    sp, pap, mp, ep, npl = (
        pools["sp"], pools["pap"], pools["mp"], pools["ep"], pools["np"],
    )
    identity = pools["identity"]
    def emit_av(pacc, js, et):
        for idx, j in enumerate(js):
            for isub in range(NSUB):
                nc.tensor.matmul(
                    pacc[:, W * isub : W * (isub + 1)],
                    lhsT=et[:, IH * idx + JCH * isub : IH * idx + JCH * (isub + 1)],
                    rhs=vT[:, W * j : W * (j + 1)],
                    start=(j == 0 and isub == 0),
                    stop=(j == NJ - 1 and isub == NSUB - 1),
                    skip_group_check=True,
                )

    for ih in range(NIH):
        # one bank holds all four (128, 65) accumulation chains
        pacc = pap.tile([JCH, NSUB * W], F32, tag="pacc", name=f"pacc{d}{ih}")
        pending = []  # AV runs LAG score-groups behind: exp is always ready
        for jg in range((NJ + GRP - 1) // GRP):
            js = list(range(GRP * jg, min(GRP * (jg + 1), NJ)))
            ps = sp.tile([JCH, GRP * IH], F32, tag="ps", name=f"ps{d}{ih}{jg}")
            for idx, j in enumerate(js):
                nc.tensor.matmul(
                    ps[:, IH * idx : IH * (idx + 1)],
                    lhsT=fa[j // JPS][:, JCH * (j % JPS) : JCH * (j % JPS + 1)],
                    rhs=qq[:, IH * ih : IH * (ih + 1)],
                    start=True,
                    stop=True,
                )
            et = ep.tile([JCH, GRP * IH], F16, tag="exp", name=f"et{d}{ih}{jg}")
            nc.scalar.activation(
                et[:, : IH * len(js)],
                ps[:, : IH * len(js)],
                mybir.ActivationFunctionType.Exp,
            )
            pending.append((pacc, js, et))
            if len(pending) > LAG:
                emit_av(*pending.pop(0))
        for args in pending:
            emit_av(*args)
        # normalize per query (partition): r = 1 / denom-column
        r4 = npl.tile([JCH, NSUB], F32, tag="r4", name=f"r4{d}{ih}")
        nc.vector.reciprocal(
            r4[:].rearrange("p (i w) -> p i w", w=1),
            pacc[:].rearrange("p (i w) -> p i w", w=W)[:, :, C : C + 1],
        )
        attT = npl.tile([JCH, NSUB * C], F32, tag="attT", name=f"attT{d}{ih}")
        for isub in range(NSUB):
            nc.vector.tensor_scalar_mul(
                attT[:, C * isub : C * (isub + 1)],
                pacc[:, W * isub : W * isub + C],
                r4[:, isub : isub + 1],
            )
        # transpose each (128, 64) subblock back to channel-major
        pt = mp.tile([C, IH], F32, tag="mp", name=f"pt{d}{ih}")
        for isub in range(NSUB):
            nc.tensor.transpose(
                pt[:, JCH * isub : JCH * (isub + 1)],
                attT[:, C * isub : C * (isub + 1)],
                identity[:],
            )
        nc.vector.tensor_copy(fused_t[ih][C * d : C * (d + 1), :], pt[:])


def _build_program(
    ctx, tc, ct, mri, qsrc_ct, qsrc_mri, wqq, wv, woT, bo, ident, out
):
    nc = tc.nc
    wpool = ctx.enter_context(tc.tile_pool(name="wpool", bufs=1))
    featp = ctx.enter_context(tc.tile_pool(name="feat", bufs=2 * NFS))
    pools = {
        "qp": ctx.enter_context(tc.tile_pool(name="qp", bufs=2)),
        "vp": ctx.enter_context(tc.tile_pool(name="vp", bufs=2)),
        "ep": ctx.enter_context(tc.tile_pool(name="ep", bufs=4)),
        "np": ctx.enter_context(tc.tile_pool(name="npool", bufs=2)),
        "sp": ctx.enter_context(
            tc.tile_pool(name="spsum", bufs=3, space="PSUM")
        ),
        "pap": ctx.enter_context(
            tc.tile_pool(name="paccp", bufs=1, space="PSUM")
        ),
        "mp": ctx.enter_context(tc.tile_pool(name="mpsum", bufs=1, space="PSUM")),
    }
    fp = ctx.enter_context(tc.tile_pool(name="fusedp", bufs=2))
    op = ctx.enter_context(tc.tile_pool(name="outp", bufs=2))

    wqq_sb = wpool.tile([W, 2 * W], F16, name="wqq_sb")
    nc.sync.dma_start(wqq_sb[:], wqq[:])
    wv_sb = wpool.tile([W, 2 * W], F16, name="wv_sb")
    nc.sync.dma_start(wv_sb[:], wv[:])
    woT_sb = wpool.tile([2 * C, C], F32, name="woT_sb")
    nc.sync.dma_start(woT_sb[:], woT[:])
    bo_sb = wpool.tile([C, 1], F32, name="bo_sb")
    nc.sync.dma_start(bo_sb[:], bo[:])
    ident_sb = wpool.tile([JCH, JCH], F32, name="ident_sb")
    nc.sync.dma_start(ident_sb[:], ident[:])
    pools["identity"] = ident_sb

    fused_t = [
        fp.tile([2 * C, IH], F32, tag="fused", name=f"fused{ih}")
        for ih in range(NIH)
    ]

    # tiny query-source DMAs go first so they don't queue behind the 2 MB
    # of feature DMAs (HWDGE queues are FIFO); split in halves so the first
    # projection can start as soon as half arrives
    qsc = pools["qp"].tile([W, NQ], F16, tag="qsrc", name="qsc")
    qsm = pools["qp"].tile([W, NQ], F16, tag="qsrc", name="qsm")
    for h in range(NIH):
        nc.sync.dma_start(
            qsc[:, IH * h : IH * (h + 1)], qsrc_ct[:, IH * h : IH * (h + 1)]
        )
        nc.sync.dma_start(
            qsm[:, IH * h : IH * (h + 1)], qsrc_mri[:, IH * h : IH * (h + 1)]
        )

    # mri side first: it feeds direction 0 (CT queries over MRI K/V)
    fs_mri = _emit_feat_load(nc, featp, mri, "fsm", "fmri")
    fs_ct = _emit_feat_load(nc, featp, ct, "fsc", "fct")
    qq_d0 = _emit_qq_proj(nc, pools, wqq_sb, qsc, 0 * W, "qq_d0")
    vT_mri = _emit_v_proj(nc, pools, wv_sb, fs_mri, 0 * W, "vT_mri")

    # direction 0 while CT projections stream in behind it
    _emit_attention(nc, pools, fs_mri, qq_d0, vT_mri, fused_t, 0)

    qq_d1 = _emit_qq_proj(nc, pools, wqq_sb, qsm, 1 * W, "qq_d1")
    vT_ct = _emit_v_proj(nc, pools, wv_sb, fs_ct, 1 * W, "vT_ct")

    _emit_attention(nc, pools, fs_ct, qq_d1, vT_ct, fused_t, 1)

    for ih in range(NIH):
        po = pools["mp"].tile([C, IH], F32, tag="mp", name=f"po{ih}")
        nc.tensor.matmul(
            po[:], lhsT=woT_sb[:], rhs=fused_t[ih][:], start=True, stop=True
        )
        ot = op.tile([C, IH], F32, tag="ot", name=f"ot{ih}")
        nc.vector.tensor_scalar_add(ot[:], po[:], bo_sb[:])
        nc.sync.dma_start(out[:, IH * ih : IH * (ih + 1)], ot[:])


def build_bass():
    nc = bacc.Bacc("TRN2", target_bir_lowering=False, debug=False)
    ct = nc.dram_tensor("ct_feat", [W, N], F16, kind="ExternalInput").ap()
    mri = nc.dram_tensor("mri_feat", [W, N], F16, kind="ExternalInput").ap()
    qsrc_ct = nc.dram_tensor("qsrc_ct", [W, NQ], F16, kind="ExternalInput").ap()
    qsrc_mri = nc.dram_tensor("qsrc_mri", [W, NQ], F16, kind="ExternalInput").ap()
    wqq = nc.dram_tensor("wqq", [W, 2 * W], F16, kind="ExternalInput").ap()
    wv = nc.dram_tensor("wv", [W, 2 * W], F16, kind="ExternalInput").ap()
    woT = nc.dram_tensor("woT", [2 * C, C], F32, kind="ExternalInput").ap()
    bo = nc.dram_tensor("bo", [C, 1], F32, kind="ExternalInput").ap()
    ident = nc.dram_tensor("ident", [JCH, JCH], F32, kind="ExternalInput").ap()
    out = nc.dram_tensor("out", [C, NQ], F32, kind="ExternalOutput").ap()

    with tile.TileContext(nc) as tc, ExitStack() as ctx:
        _build_program(
            ctx, tc, ct, mri, qsrc_ct, qsrc_mri, wqq, wv, woT, bo, ident, out
        )
    nc.compile()
    return nc


def _aug(w, b):
    # (out,in) weight + (out,) bias -> lhsT-ready [w.T; b] of shape (in+1, out)
    return np.concatenate(
        [np.asarray(w, np.float32).T, np.asarray(b, np.float32)[None, :]], axis=0
    )


def _wv_pack(w, b):
    # (65, 65): [[wv.T; bv] | e_last]: extra column accumulates the denominator
    m = np.zeros((W, W), np.float32)
    m[:, :C] = _aug(w, b)
    m[C, C] = 1.0
    return m


def _wkb_pack(w, b):
    # (64, 65): [wk | bk] -- K projection folded into the query side
    return np.concatenate(
        [np.asarray(w, np.float32), np.asarray(b, np.float32)[:, None]], axis=1
    )


def prepare_inputs(inputs):
    scale = np.float32(1.0 / np.sqrt(C))
    ct = np.asarray(inputs["ct_features"], np.float32).reshape(C, N)
    mri = np.asarray(inputs["mri_features"], np.float32).reshape(C, N)
    ones = np.ones((1, N), np.float32)
    ct_aug = np.concatenate([ct, ones], axis=0).astype(np.float16)
    mri_aug = np.concatenate([mri, ones], axis=0).astype(np.float16)
    wq_ct = _aug(np.asarray(inputs["wq_ct"]) * scale, np.asarray(inputs["bq_ct"]) * scale)
    wq_mri = _aug(np.asarray(inputs["wq_mri"]) * scale, np.asarray(inputs["bq_mri"]) * scale)
    # compose Q projection with the query-side-folded K projection (fp32 host
    # matmul, rounded to fp16 once): q''_d = (Wq_aug @ [Wk|bk])^T @ qsrc_aug
    wqq = np.concatenate(
        [wq_ct @ _wkb_pack(inputs["wk_mri"], inputs["bk_mri"]),
         wq_mri @ _wkb_pack(inputs["wk_ct"], inputs["bk_ct"])],
        axis=1,
    ).astype(np.float16)
    wv = np.concatenate(
        [_wv_pack(inputs["wv_mri"], inputs["bv_mri"]),
         _wv_pack(inputs["wv_ct"], inputs["bv_ct"])],
        axis=1,
    ).astype(np.float16)
    woT = np.ascontiguousarray(np.asarray(inputs["wo"], np.float32).T)
    bo = np.ascontiguousarray(np.asarray(inputs["bo"], np.float32)[:, None])
    ident = np.eye(JCH, dtype=np.float32)

    in_maps = []
    for i in range(NCORES):
        sl = slice(NQ * i, NQ * (i + 1))
        in_maps.append(
            {
                "ct_feat": ct_aug,
                "mri_feat": mri_aug,
                "qsrc_ct": np.ascontiguousarray(ct_aug[:, sl]),
                "qsrc_mri": np.ascontiguousarray(mri_aug[:, sl]),
                "wqq": wqq,
                "wv": wv,
                "woT": woT,
                "bo": bo,
                "ident": ident,
            }
        )
    return in_maps


def assemble_output(results):
    out = np.concatenate([results[i]["out"] for i in range(NCORES)], axis=1)
    return out.reshape(1, C, 8, 32, 32)


_NC_CACHE = None


def _get_nc():
    global _NC_CACHE
    if _NC_CACHE is None:
        _NC_CACHE = build_bass()
    return _NC_CACHE


def kernel(**inputs):
    nc = _get_nc()
    in_maps = prepare_inputs(inputs)
    res = run_bass_kernel_spmd(nc, in_maps, list(range(NCORES)))
    return assemble_output(res.results)


if __name__ == "__main__":
    nc = build_bass()
    print("built OK")



# revision 16
# speedup vs baseline: 1.2828x; 1.2828x over previous
"""Trainium2 Bass kernel for CrossModalAttention (restructured v2).

Reference computation (B=1, C=64, N=8192 voxels): two cross-attention
directions (CT queries over MRI keys/values and vice versa), each with an
8192x8192 attention matrix, fused output projection.

Sharding: each of the 8 cores owns 1024 query voxels for BOTH directions,
computes K/V over the full sequence locally and produces its own (64, 1024)
slice of the output. No collectives; the host concatenates the 8 slices.

Key structural ideas (vs the 233us baseline, which was LDWEIGHTS-bound on
1024 tiny AV matmuls and ScalarE-bound on exp):

1. K-side bias dropped (softmax shift invariance) -> score contraction is
   exactly 64, so score matmuls ROW-TILE: feature chunk 2p sits in SBUF
   partitions 0-63, chunk 2p+1 in partitions 64-127 (host packs this
   layout), and two concurrent matmuls on PE row-groups (0,0)/(64,0)
   compute both chunks' scores in one 512-cycle span. qq (and wv) are
   duplicated into both partition halves so each row-tile streams its own
   copy.
2. V projection piggybacks on the score loop (same lhsT feature chunk), so
   its weight loads amortize; V bias is folded into the output bias on the
   host (exactly: softmax weights sum to 1 after normalization).
3. AV is flipped to out=(c,i): lhsT = vT chunk (128j x 65, tiny 65-col
   weight load), rhs = exp tile (128j x 512i, full 512-col stream),
   accumulated over all 64 j-chunks into one PSUM bank. The 65th vT column
   is ones, so partition 64 of the accumulator collects the softmax
   denominator for free.
4. exp is split across engines: ScalarE runs exact table exp for ~53% of
   chunk pairs; the DVE computes the rest with a one-op Schraudolph
   approximation (y = s*1024/ln2 + B as int16, bitcast to fp16 =~ exp(s),
   ~3% sawtooth error). Emulated end-to-end on the reference data this
   costs ~1.8e-3 relative error (gate is 2e-2); it halves the exp
   wall-time, which otherwise binds at ~153us.
5. Normalization without transposes: reciprocal_approx_fast on the
   denominator row, DMA partition-broadcast of the (1,512) reciprocal to
   (64,512), one DVE multiply into the fp16 fused tile. Final projection
   contracts the two directions' fused tiles with two accumulating K=64
   matmuls.

Precision: all PE operands fp16 (fp32 accumulation in PSUM); exp tiles
fp16; normalize/final-bias fp32. Measured end-to-end error ~1.9e-3.
"""

from contextlib import ExitStack

import numpy as np

import concourse.bass as bass
import concourse.mybir as mybir
import concourse.tile as tile
from concourse import bacc
from concourse.bass_utils import run_bass_kernel_spmd

F32 = mybir.dt.float32
F16 = mybir.dt.float16
I16 = mybir.dt.int16
I32 = mybir.dt.int32

C = 64          # channels
N = 8192        # voxels (8*32*32)
NCORES = 8
NQ = N // NCORES      # 1024 queries per core
IH = 512              # query block (PSUM bank width in f32)
NIH = NQ // IH        # 2
JCH = 128             # key chunk
NJ = N // JCH         # 64 chunks per direction
NP = NJ // 2          # 32 chunk pairs (row-tiled)
W = C + 1             # 65: vT columns (64 v-channels + denominator ones)
NFS = 4               # feature DMA subtiles
FSW = (N // 2) // NFS  # 1024 packed cols per subtile (packed width is N/2)
PPS = FSW // JCH      # 8 pairs per feature subtile
LAG = 4               # pairs the AV matmuls trail the exp stage by
# v-projection batch: 8 pairs share one (128, 1024) PSUM tile; row-tile A
# outputs fill bank 0, row-tile B outputs bank 1 -- the two concurrent
# row-group matmuls must never write the same PSUM bank (device fault).
VB = 8

# Schraudolph fast-exp constants (fp16 bitcast). DVE converts fp32->int16
# by truncation (verified in sim), hence the +0.5.
EXPA = float(1024.0 / np.log(2.0))
EXPB = float(15 * 1024 - 44 + 0.5)

# pattern: pair p of (dir, ih) goes to ScalarE iff (idx*17)%32 < 17 where
# idx spreads assignments evenly; ~17/32 = 53% on ScalarE.
def _use_scalar_exp(d, ih, p):
    return (p * 17) % 32 < 17


def _build_program(ctx, tc, feat_dram, qsrc_dram, wqq, wv, woT, bo, out, dbg=None):
    nc = tc.nc
    wpool = ctx.enter_context(tc.tile_pool(name="wpool", bufs=1))
    featp = ctx.enter_context(tc.tile_pool(name="featp", bufs=NFS))
    qp = ctx.enter_context(tc.tile_pool(name="qp", bufs=2))
    vp = ctx.enter_context(tc.tile_pool(name="vp", bufs=2))
    ep = ctx.enter_context(tc.tile_pool(name="ep", bufs=LAG + 2))
    fp = ctx.enter_context(tc.tile_pool(name="fp", bufs=4))
    rp = ctx.enter_context(tc.tile_pool(name="rp", bufs=2))
    op = ctx.enter_context(tc.tile_pool(name="op", bufs=2))
    sp = ctx.enter_context(tc.tile_pool(name="sp", bufs=2, space="PSUM"))
    pap = ctx.enter_context(tc.tile_pool(name="pap", bufs=1, space="PSUM"))
    pvp = ctx.enter_context(tc.tile_pool(name="pvp", bufs=1, space="PSUM"))
    mp = ctx.enter_context(tc.tile_pool(name="mp", bufs=1, space="PSUM"))

    # weights + query sources first so they don't queue behind features
    wqq_sb = wpool.tile([W, 2 * JCH], F16, name="wqq_sb")
    nc.sync.dma_start(wqq_sb[:], wqq[:])
    wv_sb = wpool.tile([JCH, JCH], F16, name="wv_sb")
    nc.sync.dma_start(wv_sb[:], wv[:])
    woT_sb = wpool.tile([C, 2 * C], F16, name="woT_sb")
    nc.sync.dma_start(woT_sb[:], woT[:])
    bo_sb = wpool.tile([C, 1], F32, name="bo_sb")
    nc.sync.dma_start(bo_sb[:], bo[:])

    qsrc_sb = []
    for d in range(2):
        t = qp.tile([W, NQ], F16, tag="qsrc", name=f"qsrc{d}")
        for h in range(NIH):
            nc.sync.dma_start(
                t[:, IH * h : IH * (h + 1)], qsrc_dram[d][:, IH * h : IH * (h + 1)]
            )
        qsrc_sb.append(t)

    # features, packed (128, 4096): kv modality of direction 0 first
    feat_sb = [[], []]
    for d in range(2):
        for s in range(NFS):
            t = featp.tile([JCH, FSW], F16, tag=f"f{d}", name=f"feat{d}_{s}")
            nc.sync.dma_start(t[:], feat_dram[d][:, FSW * s : FSW * (s + 1)])
            feat_sb[d].append(t)

    # warm the exp activation table before the hot loop
    warm = wpool.tile([1, 8], F32, name="warm")
    nc.vector.memset(warm[:], 0.0)
    warm2 = wpool.tile([1, 8], F16, name="warm2")
    nc.scalar.activation(warm2[:], warm[:], mybir.ActivationFunctionType.Exp)

    # lhsT for the denominator broadcast matmul lives at partition 64 so its
    # base matches the reciprocal row (PSUM partition 64).
    ones_w = wpool.tile([W, C], F32, name="ones_w")
    nc.vector.memset(ones_w[C : C + 1, :], 1.0)

    def emit_qq(d):
        qq = qp.tile([JCH, NQ], F16, tag="qq", name=f"qq{d}")
        for h in range(NIH):
            pq = mp.tile([JCH, IH], F32, tag="mp", name=f"pqq{d}{h}")
            nc.tensor.matmul(
                pq[:],
                lhsT=wqq_sb[:, JCH * d : JCH * (d + 1)],
                rhs=qsrc_sb[d][:, IH * h : IH * (h + 1)],
                start=True,
                stop=True,
            )
            nc.vector.tensor_copy(qq[:, IH * h : IH * (h + 1)], pq[:])
        return qq

    fused = [
        [fp.tile([C, IH], F16, tag="fused", name=f"fused{d}{ih}") for ih in range(NIH)]
        for d in range(2)
    ]

    for d in range(2):
        qq = emit_qq(d)
        if dbg is not None and d == 0:
            dbg["_qq0_t"] = qq
        fs = feat_sb[d]
        vT = vp.tile([JCH, NJ * W], F16, tag="vT", name=f"vT{d}")
        if dbg is not None and d == 0:
            dbg["_vT0_t"] = vT
        vT3 = vT.rearrange("p (j w) -> p j w", w=W)
        nc.vector.memset(vT3[:, :, C : C + 1], 1.0)

        for ih in range(NIH):
            pacc = pap.tile([W, IH], F32, tag="pacc", name=f"pacc{d}{ih}")
            pending = []

            def emit_av(p, et):
                for half in range(2):
                    j = 2 * p + half
                    nc.tensor.matmul(
                        pacc[:],
                        lhsT=vT[:, W * j : W * (j + 1)],
                        rhs=et[:, IH * half : IH * (half + 1)],
                        start=(j == 0),
                        stop=(j == NJ - 1),
                        skip_group_check=True,
                    )

            pv = None
            for p in range(NP):
                f = fs[p // PPS]
                ca = JCH * (p % PPS)
                ps = sp.tile([JCH, 2 * IH], F32, tag="ps", name=f"ps{d}{ih}{p}")
                for half in range(2):
                    nc.tensor.matmul(
                        ps[:, IH * half : IH * (half + 1)],
                        lhsT=f[C * half : C * (half + 1), ca : ca + JCH],
                        rhs=qq[C * half : C * (half + 1), IH * ih : IH * (ih + 1)],
                        start=True,
                        stop=True,
                    )
                if ih == 0:
                    # v-projection shares the score weight loads
                    if p % VB == 0:
                        pv = pvp.tile([JCH, 2 * IH], F32, tag="pv", name=f"pv{d}{p}")
                    for half in range(2):
                        # row-tile A fills PSUM bank 0, B fills bank 1: the
                        # two concurrent matmuls must not share a bank
                        nc.tensor.matmul(
                            pv[:, IH * half + C * (p % VB) : IH * half + C * (p % VB + 1)],
                            lhsT=f[C * half : C * (half + 1), ca : ca + JCH],
                            rhs=wv_sb[C * half : C * (half + 1), C * d : C * (d + 1)],
                            start=True,
                            stop=True,
                        )
                    if p % (VB // 2) == VB // 2 - 1:
                        # copy half a batch (4 pairs) so the copy is emitted
                        # before the AV matmuls that read it (LAG pairs later)
                        b = p // VB
                        sub = (p % VB) // (VB // 2)
                        HB = VB // 2
                        vT4 = vT.rearrange(
                            "p (b j8 two w) -> p b j8 two w", two=2, w=W, j8=VB
                        )
                        for half in range(2):
                            dst = vT4[:, b, HB * sub : HB * (sub + 1), half, 0:C]
                            vsrc = pv[
                                :, IH * half + C * HB * sub : IH * half + C * HB * (sub + 1)
                            ].rearrange("p (j c) -> p j c", c=C)
                            nc.vector.tensor_copy(dst, vsrc)

                et = ep.tile([JCH, 2 * IH], F16, tag="et", name=f"et{d}{ih}{p}")
                if _use_scalar_exp(d, ih, p):
                    nc.scalar.activation(
                        et[:], ps[:], mybir.ActivationFunctionType.Exp
                    )
                else:
                    nc.vector.tensor_scalar(
                        et.bitcast(I16)[:],
                        ps[:],
                        EXPA,
                        EXPB,
                        mybir.AluOpType.mult,
                        mybir.AluOpType.add,
                    )
                pending.append((p, et))
                if len(pending) > LAG:
                    emit_av(*pending.pop(0))
            for args in pending:
                emit_av(*args)

            # normalize: r = 1/denom (row 64), broadcast via DMA, multiply
            # reciprocal of the denominator row via bitcast seed + one
            # Newton step (standard DVE ops only: the custom-DVE table is
            # not loaded on this execution path, so reciprocal_approx_fast
            # returns garbage on HW)
            r_scr = rp.tile([W, 4 * IH], F32, tag="rrow", name=f"rr{d}{ih}")
            r0v = r_scr[C : C + 1, 0:IH]
            tv = r_scr[C : C + 1, IH : 2 * IH]
            uv = r_scr[C : C + 1, 2 * IH : 3 * IH]
            r1v = r_scr[C : C + 1, 3 * IH : 4 * IH]
            nc.vector.tensor_scalar(
                uv.bitcast(I32),
                pacc[C : C + 1, :].bitcast(I32),
                -1,
                None,
                mybir.AluOpType.bitwise_xor,
            )
            nc.vector.tensor_scalar(
                r0v.bitcast(I32),
                uv.bitcast(I32),
                0x7EF311C3 + 1,
                None,
                mybir.AluOpType.add,
            )
            nc.vector.tensor_mul(tv, pacc[C : C + 1, :], r0v)
            nc.vector.tensor_scalar(
                uv, tv, -1.0, 2.0, mybir.AluOpType.mult, mybir.AluOpType.add
            )
            nc.vector.tensor_mul(r1v, r0v, uv)
            r_row = r_scr[:, 3 * IH : 4 * IH]
            if dbg is not None and d == 0 and ih == 0:
                den_s = rp.tile([W, IH], F32, tag="dens", name="den_s", bufs=1)
                nc.vector.tensor_copy(den_s[:], pacc[:])
                nc.sync.dma_start(dbg["den0"][:], den_s[:])
                nc.sync.dma_start(dbg["rr0"][:], r_row[C : C + 1, :])
            rbp = mp.tile([C, IH], F32, tag="mp", name=f"rbp{d}{ih}")
            nc.tensor.matmul(
                rbp[:],
                lhsT=ones_w[C : C + 1, :],
                rhs=r_row[C : C + 1, :],
                start=True,
                stop=True,
            )
            rb = rp.tile([C, IH], F32, tag="rb", name=f"rb{d}{ih}")
            nc.scalar.copy(rb[:], rbp[:])
            if dbg is not None and d == 0 and ih == 0:
                nc.sync.dma_start(dbg["rb0"][:], rb[:])
            nc.vector.tensor_mul(fused[d][ih][:], pacc[0:C, :], rb[:])

    if dbg is not None:
        nc.sync.dma_start(dbg["qq0"][:], dbg["_qq0_t"][:])
        nc.sync.dma_start(dbg["vT0"][:], dbg["_vT0_t"][:])
        for d in range(2):
            for ih in range(NIH):
                nc.sync.dma_start(dbg[f"fu{d}{ih}"][:], fused[d][ih][:])

    for ih in range(NIH):
        po = mp.tile([C, IH], F32, tag="mp", name=f"po{ih}")
        for d in range(2):
            nc.tensor.matmul(
                po[:],
                lhsT=woT_sb[:, C * d : C * (d + 1)],
                rhs=fused[d][ih][:],
                start=(d == 0),
                stop=(d == 1),
            )
        ot = op.tile([C, IH], F32, tag="ot", name=f"ot{ih}")
        nc.vector.tensor_scalar_add(ot[:], po[:], bo_sb[:])
        nc.sync.dma_start(out[:, IH * ih : IH * (ih + 1)], ot[:])


def build_bass(debug_taps=False):
    nc = bacc.Bacc("TRN2", target_bir_lowering=False, debug=False)
    feat_mri = nc.dram_tensor("feat_mri", [JCH, N // 2], F16, kind="ExternalInput").ap()
    feat_ct = nc.dram_tensor("feat_ct", [JCH, N // 2], F16, kind="ExternalInput").ap()
    qsrc_ct = nc.dram_tensor("qsrc_ct", [W, NQ], F16, kind="ExternalInput").ap()
    qsrc_mri = nc.dram_tensor("qsrc_mri", [W, NQ], F16, kind="ExternalInput").ap()
    wqq = nc.dram_tensor("wqq", [W, 2 * JCH], F16, kind="ExternalInput").ap()
    wv = nc.dram_tensor("wv", [JCH, JCH], F16, kind="ExternalInput").ap()
    woT = nc.dram_tensor("woT", [C, 2 * C], F16, kind="ExternalInput").ap()
    bo = nc.dram_tensor("bo", [C, 1], F32, kind="ExternalInput").ap()
    out = nc.dram_tensor("out", [C, NQ], F32, kind="ExternalOutput").ap()
    dbg = None
    if debug_taps:
        dbg = {
            "den0": nc.dram_tensor("dbg_den0", [W, IH], F32, kind="ExternalOutput").ap(),
            "rr0": nc.dram_tensor("dbg_rr0", [1, IH], F32, kind="ExternalOutput").ap(),
            "rb0": nc.dram_tensor("dbg_rb0", [C, IH], F32, kind="ExternalOutput").ap(),
            "qq0": nc.dram_tensor("dbg_qq0", [JCH, NQ], F16, kind="ExternalOutput").ap(),
            "vT0": nc.dram_tensor("dbg_vT0", [JCH, NJ * W], F16, kind="ExternalOutput").ap(),
        }
        for d in range(2):
            for ih in range(NIH):
                dbg[f"fu{d}{ih}"] = nc.dram_tensor(
                    f"dbg_fu{d}{ih}", [C, IH], F16, kind="ExternalOutput"
                ).ap()

    with tile.TileContext(nc) as tc, ExitStack() as ctx:
        _build_program(
            ctx,
            tc,
            [feat_mri, feat_ct],
            [qsrc_ct, qsrc_mri],
            wqq,
            wv,
            woT,
            bo,
            out,
            dbg=dbg,
        )
    nc.compile()
    return nc


def _pack_feat(feat):
    # (64, 8192) -> (128, 4096): chunk 2p in partitions 0-63, 2p+1 in 64-127
    f = feat.reshape(C, NP, 2, JCH)
    return np.concatenate(
        [f[:, :, 0, :].reshape(C, NP * JCH), f[:, :, 1, :].reshape(C, NP * JCH)],
        axis=0,
    )


def prepare_inputs(inputs):
    scale = np.float32(1.0 / np.sqrt(C))
    ct = np.asarray(inputs["ct_features"], np.float32).reshape(C, N)
    mri = np.asarray(inputs["mri_features"], np.float32).reshape(C, N)
    ct16 = ct.astype(np.float16).astype(np.float32)
    mri16 = mri.astype(np.float16).astype(np.float32)
    feat_mri = _pack_feat(mri16).astype(np.float16)
    feat_ct = _pack_feat(ct16).astype(np.float16)

    ones = np.ones((1, N), np.float32)
    ct_aug = np.concatenate([ct, ones], axis=0).astype(np.float16)
    mri_aug = np.concatenate([mri, ones], axis=0).astype(np.float16)

    def wqq_dir(wq, bq, wk):
        m = (
            np.concatenate(
                [np.asarray(wq, np.float32).T, np.asarray(bq, np.float32)[None]], axis=0
            )
            @ np.asarray(wk, np.float32)
        ) * scale  # (65, 64)
        return np.concatenate([m, m], axis=1)  # duplicated for row-tile B

    wqq = np.concatenate(
        [
            wqq_dir(inputs["wq_ct"], inputs["bq_ct"], inputs["wk_mri"]),
            wqq_dir(inputs["wq_mri"], inputs["bq_mri"], inputs["wk_ct"]),
        ],
        axis=1,
    ).astype(np.float16)  # (65, 256)

    def wv_dir(w):
        m = np.asarray(w, np.float32).T  # (cin, cout)
        return np.concatenate([m, m], axis=0)  # (128, 64), duplicated rows

    wv = np.concatenate(
        [wv_dir(inputs["wv_mri"]), wv_dir(inputs["wv_ct"])], axis=1
    ).astype(np.float16)  # (128, 128)

    wo = np.asarray(inputs["wo"], np.float32)  # (64, 128)
    woT = np.ascontiguousarray(
        np.concatenate([wo[:, :C].T, wo[:, C:].T], axis=1)
    ).astype(np.float16)  # (64, 128): [dir0 block | dir1 block]
    bo_adj = (
        np.asarray(inputs["bo"], np.float32)
        + wo[:, :C] @ np.asarray(inputs["bv_mri"], np.float32)
        + wo[:, C:] @ np.asarray(inputs["bv_ct"], np.float32)
    )[:, None]

    in_maps = []
    for i in range(NCORES):
        sl = slice(NQ * i, NQ * (i + 1))
        in_maps.append(
            {
                "feat_mri": feat_mri,
                "feat_ct": feat_ct,
                "qsrc_ct": np.ascontiguousarray(ct_aug[:, sl]),
                "qsrc_mri": np.ascontiguousarray(mri_aug[:, sl]),
                "wqq": wqq,
                "wv": wv,
                "woT": woT,
                "bo": bo_adj,
            }
        )
    return in_maps


def assemble_output(results):
    out = np.concatenate([results[i]["out"] for i in range(NCORES)], axis=1)
    return out.reshape(1, C, 8, 32, 32)


_NC_CACHE = None


def _get_nc():
    global _NC_CACHE
    if _NC_CACHE is None:
        _NC_CACHE = build_bass()
    return _NC_CACHE


def kernel(**inputs):
    nc = _get_nc()
    in_maps = prepare_inputs(inputs)
    res = run_bass_kernel_spmd(nc, in_maps, list(range(NCORES)))
    return assemble_output(res.results)


if __name__ == "__main__":
    nc = build_bass()
    print("built OK")


# revision 17
# speedup vs baseline: 1.3662x; 1.0651x over previous
"""Trainium2 Bass kernel for CrossModalAttention (restructured v2).

Reference computation (B=1, C=64, N=8192 voxels): two cross-attention
directions (CT queries over MRI keys/values and vice versa), each with an
8192x8192 attention matrix, fused output projection.

Sharding: each of the 8 cores owns 1024 query voxels for BOTH directions,
computes K/V over the full sequence locally and produces its own (64, 1024)
slice of the output. No collectives; the host concatenates the 8 slices.

Key structural ideas (vs the 233us baseline, which was LDWEIGHTS-bound on
1024 tiny AV matmuls and ScalarE-bound on exp):

1. K-side bias dropped (softmax shift invariance) -> score contraction is
   exactly 64, so score matmuls ROW-TILE: feature chunk 2p sits in SBUF
   partitions 0-63, chunk 2p+1 in partitions 64-127 (host packs this
   layout), and two concurrent matmuls on PE row-groups (0,0)/(64,0)
   compute both chunks' scores in one 512-cycle span. qq (and wv) are
   duplicated into both partition halves so each row-tile streams its own
   copy.
2. V projection piggybacks on the score loop (same lhsT feature chunk), so
   its weight loads amortize; V bias is folded into the output bias on the
   host (exactly: softmax weights sum to 1 after normalization).
3. AV is flipped to out=(c,i): lhsT = vT chunk (128j x 65, tiny 65-col
   weight load), rhs = exp tile (128j x 512i, full 512-col stream),
   accumulated over all 64 j-chunks into one PSUM bank. The 65th vT column
   is ones, so partition 64 of the accumulator collects the softmax
   denominator for free.
4. exp is split across engines: ScalarE runs exact table exp for ~53% of
   chunk pairs; the DVE computes the rest with a one-op Schraudolph
   approximation (y = s*1024/ln2 + B as int16, bitcast to fp16 =~ exp(s),
   ~3% sawtooth error). Emulated end-to-end on the reference data this
   costs ~1.8e-3 relative error (gate is 2e-2); it halves the exp
   wall-time, which otherwise binds at ~153us.
5. Normalization without transposes: reciprocal_approx_fast on the
   denominator row, DMA partition-broadcast of the (1,512) reciprocal to
   (64,512), one DVE multiply into the fp16 fused tile. Final projection
   contracts the two directions' fused tiles with two accumulating K=64
   matmuls.

Precision: all PE operands fp16 (fp32 accumulation in PSUM); exp tiles
fp16; normalize/final-bias fp32. Measured end-to-end error ~1.9e-3.
"""

from contextlib import ExitStack

import numpy as np

import concourse.bass as bass
import concourse.mybir as mybir
import concourse.tile as tile
from concourse import bacc
from concourse.bass_utils import run_bass_kernel_spmd

F32 = mybir.dt.float32
F16 = mybir.dt.float16
I16 = mybir.dt.int16
I32 = mybir.dt.int32

C = 64          # channels
N = 8192        # voxels (8*32*32)
NCORES = 8
NQ = N // NCORES      # 1024 queries per core
IH = 512              # query block (PSUM bank width in f32)
NIH = NQ // IH        # 2
JCH = 128             # key chunk
NJ = N // JCH         # 64 chunks per direction
NP = NJ // 2          # 32 chunk pairs (row-tiled)
W = C + 1             # 65: vT columns (64 v-channels + denominator ones)
NFS = 4               # feature DMA subtiles
FSW = (N // 2) // NFS  # 1024 packed cols per subtile (packed width is N/2)
PPS = FSW // JCH      # 8 pairs per feature subtile
LAG = 4               # pairs the AV matmuls trail the exp stage by
# v-projection batch: 8 pairs share one (128, 1024) PSUM tile; row-tile A
# outputs fill bank 0, row-tile B outputs bank 1 -- the two concurrent
# row-group matmuls must never write the same PSUM bank (device fault).
VB = 8

# Schraudolph fast-exp constants (fp16 bitcast). DVE converts fp32->int16
# by truncation (verified in sim), hence the +0.5.
EXPA = float(1024.0 / np.log(2.0))
EXPB = float(15 * 1024 - 44 + 0.5)

# pattern: pair p of (dir, ih) goes to ScalarE iff (idx*17)%32 < 17 where
# idx spreads assignments evenly; ~17/32 = 53% on ScalarE.
def _use_scalar_exp(d, ih, p):
    return (p * 17) % 32 < 17


def _build_program(ctx, tc, feat_dram, qsrc_dram, wqq, wv, woT, bo, out, dbg=None):
    nc = tc.nc
    wpool = ctx.enter_context(tc.tile_pool(name="wpool", bufs=1))
    featp = ctx.enter_context(tc.tile_pool(name="featp", bufs=NFS))
    qp = ctx.enter_context(tc.tile_pool(name="qp", bufs=2))
    vp = ctx.enter_context(tc.tile_pool(name="vp", bufs=2))
    ep = ctx.enter_context(tc.tile_pool(name="ep", bufs=LAG + 2))
    fp = ctx.enter_context(tc.tile_pool(name="fp", bufs=4))
    rp = ctx.enter_context(tc.tile_pool(name="rp", bufs=2))
    op = ctx.enter_context(tc.tile_pool(name="op", bufs=2))
    sp = ctx.enter_context(tc.tile_pool(name="sp", bufs=2, space="PSUM"))
    pap = ctx.enter_context(tc.tile_pool(name="pap", bufs=2, space="PSUM"))
    pvp = ctx.enter_context(tc.tile_pool(name="pvp", bufs=1, space="PSUM"))

    # weights + query sources first so they don't queue behind features
    wqq_sb = wpool.tile([W, 2 * JCH], F16, name="wqq_sb")
    nc.sync.dma_start(wqq_sb[:], wqq[:])
    wv_sb = wpool.tile([JCH, JCH], F16, name="wv_sb")
    nc.sync.dma_start(wv_sb[:], wv[:])
    woT_sb = wpool.tile([C, 2 * C], F16, name="woT_sb")
    nc.sync.dma_start(woT_sb[:], woT[:])
    bo_sb = wpool.tile([C, 1], F32, name="bo_sb")
    nc.sync.dma_start(bo_sb[:], bo[:])

    qsrc_sb = []
    for d in range(2):
        t = qp.tile([W, NQ], F16, tag="qsrc", name=f"qsrc{d}")
        for h in range(NIH):
            nc.sync.dma_start(
                t[:, IH * h : IH * (h + 1)], qsrc_dram[d][:, IH * h : IH * (h + 1)]
            )
        qsrc_sb.append(t)

    # features, packed (128, 4096): kv modality of direction 0 first
    feat_sb = [[], []]
    for d in range(2):
        for s in range(NFS):
            t = featp.tile([JCH, FSW], F16, tag=f"f{d}", name=f"feat{d}_{s}")
            nc.sync.dma_start(t[:], feat_dram[d][:, FSW * s : FSW * (s + 1)])
            feat_sb[d].append(t)

    # warm the exp activation table before the hot loop
    warm = wpool.tile([1, 8], F32, name="warm")
    nc.vector.memset(warm[:], 0.0)
    warm2 = wpool.tile([1, 8], F16, name="warm2")
    nc.scalar.activation(warm2[:], warm[:], mybir.ActivationFunctionType.Exp)

    # lhsT for the denominator broadcast matmul lives at partition 64 so its
    # base matches the reciprocal row (PSUM partition 64).
    ones_w = wpool.tile([W, C], F32, name="ones_w")
    nc.vector.memset(ones_w[C : C + 1, :], 1.0)

    def emit_qq(d):
        qq = qp.tile([JCH, NQ], F16, tag="qq", name=f"qq{d}")
        for h in range(NIH):
            pq = pvp.tile([JCH, IH], F32, tag="pv", name=f"pqq{d}{h}")
            nc.tensor.matmul(
                pq[:],
                lhsT=wqq_sb[:, JCH * d : JCH * (d + 1)],
                rhs=qsrc_sb[d][:, IH * h : IH * (h + 1)],
                start=True,
                stop=True,
            )
            nc.vector.tensor_copy(qq[:, IH * h : IH * (h + 1)], pq[:])
        return qq

    fused = [
        [fp.tile([C, IH], F16, tag="fused", name=f"fused{d}{ih}") for ih in range(NIH)]
        for d in range(2)
    ]

    for d in range(2):
        qq = emit_qq(d)
        if dbg is not None and d == 0:
            dbg["_qq0_t"] = qq
        fs = feat_sb[d]
        vT = vp.tile([JCH, NJ * W], F16, tag="vT", name=f"vT{d}")
        if dbg is not None and d == 0:
            dbg["_vT0_t"] = vT
        vT3 = vT.rearrange("p (j w) -> p j w", w=W)
        nc.vector.memset(vT3[:, :, C : C + 1], 1.0)

        for ih in range(NIH):
            pacc = pap.tile([W, IH], F32, tag="pacc", name=f"pacc{d}{ih}")
            pending = []

            def emit_av(p, et):
                for half in range(2):
                    j = 2 * p + half
                    nc.tensor.matmul(
                        pacc[:],
                        lhsT=vT[:, W * j : W * (j + 1)],
                        rhs=et[:, IH * half : IH * (half + 1)],
                        start=(j == 0),
                        stop=(j == NJ - 1),
                        skip_group_check=True,
                    )

            pv = None
            for p in range(NP):
                f = fs[p // PPS]
                ca = JCH * (p % PPS)
                ps = sp.tile([JCH, 2 * IH], F32, tag="ps", name=f"ps{d}{ih}{p}")
                for half in range(2):
                    nc.tensor.matmul(
                        ps[:, IH * half : IH * (half + 1)],
                        lhsT=f[C * half : C * (half + 1), ca : ca + JCH],
                        rhs=qq[C * half : C * (half + 1), IH * ih : IH * (ih + 1)],
                        start=True,
                        stop=True,
                    )
                if ih == 0:
                    # v-projection shares the score weight loads
                    if p % VB == 0:
                        pv = pvp.tile([JCH, 2 * IH], F32, tag="pv", name=f"pv{d}{p}")
                    for half in range(2):
                        # row-tile A fills PSUM bank 0, B fills bank 1: the
                        # two concurrent matmuls must not share a bank
                        nc.tensor.matmul(
                            pv[:, IH * half + C * (p % VB) : IH * half + C * (p % VB + 1)],
                            lhsT=f[C * half : C * (half + 1), ca : ca + JCH],
                            rhs=wv_sb[C * half : C * (half + 1), C * d : C * (d + 1)],
                            start=True,
                            stop=True,
                        )
                    if p % (VB // 2) == VB // 2 - 1:
                        # copy half a batch (4 pairs) so the copy is emitted
                        # before the AV matmuls that read it (LAG pairs later)
                        b = p // VB
                        sub = (p % VB) // (VB // 2)
                        HB = VB // 2
                        vT4 = vT.rearrange(
                            "p (b j8 two w) -> p b j8 two w", two=2, w=W, j8=VB
                        )
                        for half in range(2):
                            dst = vT4[:, b, HB * sub : HB * (sub + 1), half, 0:C]
                            vsrc = pv[
                                :, IH * half + C * HB * sub : IH * half + C * HB * (sub + 1)
                            ].rearrange("p (j c) -> p j c", c=C)
                            nc.vector.tensor_copy(dst, vsrc)

                et = ep.tile([JCH, 2 * IH], F16, tag="et", name=f"et{d}{ih}{p}")
                if _use_scalar_exp(d, ih, p):
                    nc.scalar.activation(
                        et[:], ps[:], mybir.ActivationFunctionType.Exp
                    )
                else:
                    nc.vector.tensor_scalar(
                        et.bitcast(I16)[:],
                        ps[:],
                        EXPA,
                        EXPB,
                        mybir.AluOpType.mult,
                        mybir.AluOpType.add,
                    )
                pending.append((p, et))
                if len(pending) > LAG:
                    emit_av(*pending.pop(0))
            for args in pending:
                emit_av(*args)

            # normalize: r = 1/denom (row 64), broadcast via DMA, multiply
            # reciprocal of the denominator row via bitcast seed + one
            # Newton step (standard DVE ops only: the custom-DVE table is
            # not loaded on this execution path, so reciprocal_approx_fast
            # returns garbage on HW)
            r_scr = rp.tile([W, 4 * IH], F32, tag="rrow", name=f"rr{d}{ih}")
            r0v = r_scr[C : C + 1, 0:IH]
            tv = r_scr[C : C + 1, IH : 2 * IH]
            uv = r_scr[C : C + 1, 2 * IH : 3 * IH]
            r1v = r_scr[C : C + 1, 3 * IH : 4 * IH]
            nc.vector.tensor_scalar(
                uv.bitcast(I32),
                pacc[C : C + 1, :].bitcast(I32),
                -1,
                None,
                mybir.AluOpType.bitwise_xor,
            )
            nc.vector.tensor_scalar(
                r0v.bitcast(I32),
                uv.bitcast(I32),
                0x7EF311C3 + 1,
                None,
                mybir.AluOpType.add,
            )
            nc.vector.tensor_mul(tv, pacc[C : C + 1, :], r0v)
            nc.vector.tensor_scalar(
                uv, tv, -1.0, 2.0, mybir.AluOpType.mult, mybir.AluOpType.add
            )
            nc.vector.tensor_mul(r1v, r0v, uv)
            r_row = r_scr[:, 3 * IH : 4 * IH]
            if dbg is not None and d == 0 and ih == 0:
                den_s = rp.tile([W, IH], F32, tag="dens", name="den_s", bufs=1)
                nc.vector.tensor_copy(den_s[:], pacc[:])
                nc.sync.dma_start(dbg["den0"][:], den_s[:])
                nc.sync.dma_start(dbg["rr0"][:], r_row[C : C + 1, :])
            rbp = pvp.tile([C, IH], F32, tag="pv", name=f"rbp{d}{ih}")
            nc.tensor.matmul(
                rbp[:],
                lhsT=ones_w[C : C + 1, :],
                rhs=r_row[C : C + 1, :],
                start=True,
                stop=True,
            )
            rb = rp.tile([C, IH], F32, tag="rb", name=f"rb{d}{ih}")
            nc.scalar.copy(rb[:], rbp[:])
            if dbg is not None and d == 0 and ih == 0:
                nc.sync.dma_start(dbg["rb0"][:], rb[:])
            nc.vector.tensor_mul(fused[d][ih][:], pacc[0:C, :], rb[:])

    if dbg is not None:
        nc.sync.dma_start(dbg["qq0"][:], dbg["_qq0_t"][:])
        nc.sync.dma_start(dbg["vT0"][:], dbg["_vT0_t"][:])
        for d in range(2):
            for ih in range(NIH):
                nc.sync.dma_start(dbg[f"fu{d}{ih}"][:], fused[d][ih][:])

    for ih in range(NIH):
        po = pvp.tile([C, IH], F32, tag="pv", name=f"po{ih}")
        for d in range(2):
            nc.tensor.matmul(
                po[:],
                lhsT=woT_sb[:, C * d : C * (d + 1)],
                rhs=fused[d][ih][:],
                start=(d == 0),
                stop=(d == 1),
            )
        ot = op.tile([C, IH], F32, tag="ot", name=f"ot{ih}")
        nc.vector.tensor_scalar_add(ot[:], po[:], bo_sb[:])
        nc.sync.dma_start(out[:, IH * ih : IH * (ih + 1)], ot[:])


def build_bass(debug_taps=False):
    nc = bacc.Bacc("TRN2", target_bir_lowering=False, debug=False)
    feat_mri = nc.dram_tensor("feat_mri", [JCH, N // 2], F16, kind="ExternalInput").ap()
    feat_ct = nc.dram_tensor("feat_ct", [JCH, N // 2], F16, kind="ExternalInput").ap()
    qsrc_ct = nc.dram_tensor("qsrc_ct", [W, NQ], F16, kind="ExternalInput").ap()
    qsrc_mri = nc.dram_tensor("qsrc_mri", [W, NQ], F16, kind="ExternalInput").ap()
    wqq = nc.dram_tensor("wqq", [W, 2 * JCH], F16, kind="ExternalInput").ap()
    wv = nc.dram_tensor("wv", [JCH, JCH], F16, kind="ExternalInput").ap()
    woT = nc.dram_tensor("woT", [C, 2 * C], F16, kind="ExternalInput").ap()
    bo = nc.dram_tensor("bo", [C, 1], F32, kind="ExternalInput").ap()
    out = nc.dram_tensor("out", [C, NQ], F32, kind="ExternalOutput").ap()
    dbg = None
    if debug_taps:
        dbg = {
            "den0": nc.dram_tensor("dbg_den0", [W, IH], F32, kind="ExternalOutput").ap(),
            "rr0": nc.dram_tensor("dbg_rr0", [1, IH], F32, kind="ExternalOutput").ap(),
            "rb0": nc.dram_tensor("dbg_rb0", [C, IH], F32, kind="ExternalOutput").ap(),
            "qq0": nc.dram_tensor("dbg_qq0", [JCH, NQ], F16, kind="ExternalOutput").ap(),
            "vT0": nc.dram_tensor("dbg_vT0", [JCH, NJ * W], F16, kind="ExternalOutput").ap(),
        }
        for d in range(2):
            for ih in range(NIH):
                dbg[f"fu{d}{ih}"] = nc.dram_tensor(
                    f"dbg_fu{d}{ih}", [C, IH], F16, kind="ExternalOutput"
                ).ap()

    with tile.TileContext(nc) as tc, ExitStack() as ctx:
        _build_program(
            ctx,
            tc,
            [feat_mri, feat_ct],
            [qsrc_ct, qsrc_mri],
            wqq,
            wv,
            woT,
            bo,
            out,
            dbg=dbg,
        )
    nc.compile()
    return nc


def _pack_feat(feat):
    # (64, 8192) -> (128, 4096): chunk 2p in partitions 0-63, 2p+1 in 64-127
    f = feat.reshape(C, NP, 2, JCH)
    return np.concatenate(
        [f[:, :, 0, :].reshape(C, NP * JCH), f[:, :, 1, :].reshape(C, NP * JCH)],
        axis=0,
    )


def prepare_inputs(inputs):
    scale = np.float32(1.0 / np.sqrt(C))
    ct = np.asarray(inputs["ct_features"], np.float32).reshape(C, N)
    mri = np.asarray(inputs["mri_features"], np.float32).reshape(C, N)
    ct16 = ct.astype(np.float16).astype(np.float32)
    mri16 = mri.astype(np.float16).astype(np.float32)
    feat_mri = _pack_feat(mri16).astype(np.float16)
    feat_ct = _pack_feat(ct16).astype(np.float16)

    ones = np.ones((1, N), np.float32)
    ct_aug = np.concatenate([ct, ones], axis=0).astype(np.float16)
    mri_aug = np.concatenate([mri, ones], axis=0).astype(np.float16)

    def wqq_dir(wq, bq, wk):
        m = (
            np.concatenate(
                [np.asarray(wq, np.float32).T, np.asarray(bq, np.float32)[None]], axis=0
            )
            @ np.asarray(wk, np.float32)
        ) * scale  # (65, 64)
        return np.concatenate([m, m], axis=1)  # duplicated for row-tile B

    wqq = np.concatenate(
        [
            wqq_dir(inputs["wq_ct"], inputs["bq_ct"], inputs["wk_mri"]),
            wqq_dir(inputs["wq_mri"], inputs["bq_mri"], inputs["wk_ct"]),
        ],
        axis=1,
    ).astype(np.float16)  # (65, 256)

    def wv_dir(w):
        m = np.asarray(w, np.float32).T  # (cin, cout)
        return np.concatenate([m, m], axis=0)  # (128, 64), duplicated rows

    wv = np.concatenate(
        [wv_dir(inputs["wv_mri"]), wv_dir(inputs["wv_ct"])], axis=1
    ).astype(np.float16)  # (128, 128)

    wo = np.asarray(inputs["wo"], np.float32)  # (64, 128)
    woT = np.ascontiguousarray(
        np.concatenate([wo[:, :C].T, wo[:, C:].T], axis=1)
    ).astype(np.float16)  # (64, 128): [dir0 block | dir1 block]
    bo_adj = (
        np.asarray(inputs["bo"], np.float32)
        + wo[:, :C] @ np.asarray(inputs["bv_mri"], np.float32)
        + wo[:, C:] @ np.asarray(inputs["bv_ct"], np.float32)
    )[:, None]

    in_maps = []
    for i in range(NCORES):
        sl = slice(NQ * i, NQ * (i + 1))
        in_maps.append(
            {
                "feat_mri": feat_mri,
                "feat_ct": feat_ct,
                "qsrc_ct": np.ascontiguousarray(ct_aug[:, sl]),
                "qsrc_mri": np.ascontiguousarray(mri_aug[:, sl]),
                "wqq": wqq,
                "wv": wv,
                "woT": woT,
                "bo": bo_adj,
            }
        )
    return in_maps


def assemble_output(results):
    out = np.concatenate([results[i]["out"] for i in range(NCORES)], axis=1)
    return out.reshape(1, C, 8, 32, 32)


_NC_CACHE = None


def _get_nc():
    global _NC_CACHE
    if _NC_CACHE is None:
        _NC_CACHE = build_bass()
    return _NC_CACHE


def kernel(**inputs):
    nc = _get_nc()
    in_maps = prepare_inputs(inputs)
    res = run_bass_kernel_spmd(nc, in_maps, list(range(NCORES)))
    return assemble_output(res.results)


if __name__ == "__main__":
    nc = build_bass()
    print("built OK")
